# revision 34
# baseline (speedup 1.0000x reference)
"""ChunkedLinearAttention Trainium2 kernel — 8-core SPMD.

Sharding: core c -> batch b = c//2, head-half hh = c%2 (8 of 16 heads).
Each core computes qkv projection for its heads, chunked local attention +
cross-chunk linear term, and a row-sharded out-projection producing a partial
[4096, 1024] output; host sums the two half partials per batch element.

All matmuls in bf16 (fp32 accumulate in PSUM).  Layouts:
  xT    [1024, 4096]  x[b] transposed (host-side), bf16
  qkT   [cols, tok]   computed on PE: lhsT=Wqk tile, rhs=xT tile
  v     [tok, vcols]  computed on PE: lhsT=xT tile, rhs=Wv
  per head-pair: qT/kT [128(2 heads x 64 dims), 8 chunks, 64 tok]
  scores S [128(2 heads x 64 q), 8, 64 k] via per-chunk matmuls packed with
  tile_position (head A rows 0-63 / head B rows 64-127)
  out_localT [128(2 heads x 64 dims), 512 tok] accumulated in PSUM, with the
  cross term added via a [K=8 chunks] matmul against cum_v.
"""

import sys

if "/opt/trn_rl_repo" not in sys.path:
    sys.path.insert(0, "/opt/trn_rl_repo")

import numpy as np
import ml_dtypes

import concourse.bacc as bacc
import concourse.tile as tile
import concourse.mybir as mybir
from concourse.bass_utils import run_bass_kernel_spmd

F32 = mybir.dt.float32
BF16 = mybir.dt.bfloat16
AFT = mybir.ActivationFunctionType

DIM, H, D, CS = 1024, 16, 64, 64
SCALE = D ** -0.5
B, N = 4, 4096
NBLK, TB = 8, 512          # token blocks
NC_CHUNKS = 8              # chunks per block
HPC = 8                    # heads per core
NPAIR = 4                  # head pairs per core
N_CORES = 8

_cache = {}


def _build():
    nc = bacc.Bacc("TRN2", target_bir_lowering=False, debug=False,
                   num_devices=N_CORES)

    # ---- DRAM I/O -------------------------------------------------------
    xT_d = nc.dram_tensor("xT", [DIM, N], BF16, kind="ExternalInput")
    wqk_d = nc.dram_tensor("wqk", [DIM, 1024], BF16, kind="ExternalInput")
    wv_d = nc.dram_tensor("wv", [DIM, 512], BF16, kind="ExternalInput")
    wout_d = nc.dram_tensor("wout", [512, DIM], BF16, kind="ExternalInput")
    ident_d = nc.dram_tensor("ident", [128, 128], BF16, kind="ExternalInput")
    maskqk_d = nc.dram_tensor("maskqk", [128, 2048], BF16, kind="ExternalInput")
    mean_d = nc.dram_tensor("meanm", [128, 32], BF16, kind="ExternalInput")
    tri9_d = nc.dram_tensor("tri9", [8, 33], BF16, kind="ExternalInput")
    ones19_d = nc.dram_tensor("ones19", [1, 33], BF16, kind="ExternalInput")
    bdmask_d = nc.dram_tensor("bdmask", [128, 512], BF16, kind="ExternalInput")
    # bf16 partials: host sums the two half-contraction partials in f32.
    out_d = nc.dram_tensor("out", [N, DIM], BF16, kind="ExternalOutput")

    with tile.TileContext(nc) as tc:
        with (
            tc.tile_pool(name="const", bufs=1) as cpool,
            tc.tile_pool(name="persist", bufs=1) as ppool,
            tc.tile_pool(name="work", bufs=2) as wpool,
            tc.tile_pool(name="psq", bufs=4, space="PSUM") as psq,
            tc.tile_pool(name="psa", bufs=4, space="PSUM") as psa,
        ):
            # ---- constants / weights into SBUF --------------------------
            # DMA issue order matters: the Sync engine serializes issue at
            # ~0.6us per DMA and transfers drain in issue order, so
            # block-0's dependencies (wqk + xT block 0) go FIRST; the small
            # constants are only needed mid-block and come later.  ident
            # leads so PE warm-up matmuls can run during the ~12us
            # bandwidth-bound weight load (HAM un-throttles after ~3.4us
            # of activity).
            ident = cpool.tile([128, 128], BF16, name="ident")
            nc.sync.dma_start(ident[:], ident_d[:])
            # Each DMA queue sustains only ~16B/ns, so a whole 256KB tile
            # on one queue takes ~16us.  Split the block-0-critical tiles
            # in half and issue from two engines in parallel so the
            # startup is bound by aggregate (not per-queue) bandwidth.
            wqk = []
            for i in range(8):
                t = ppool.tile([128, 1024], BF16, name=f"wqk{i}", tag=f"wqk{i}")
                nc.sync.dma_start(t[:], wqk_d[i * 128:(i + 1) * 128, :])
                wqk.append(t)
            xT = [ppool.tile([128, N], BF16, name=f"xT{i}", tag=f"xT{i}")
                  for i in range(8)]
            for i in range(8):
                nc.sync.dma_start(xT[i][:, 0:TB], xT_d[i * 128:(i + 1) * 128, 0:TB])
            wv = []
            for i in range(8):
                t = ppool.tile([128, 512], BF16, name=f"wv{i}", tag=f"wv{i}")
                nc.sync.dma_start(t[:], wv_d[i * 128:(i + 1) * 128, :])
                wv.append(t)
            meanm = cpool.tile([128, 32], BF16, name="meanm")
            nc.sync.dma_start(meanm[:], mean_d[:])
            tri9 = cpool.tile([8, 33], BF16, name="tri9")
            nc.sync.dma_start(tri9[:], tri9_d[:])
            ones19 = cpool.tile([1, 33], BF16, name="ones19")
            nc.sync.dma_start(ones19[:], ones19_d[:])
            bdmask = cpool.tile([128, 512], BF16, name="bdmask")
            nc.sync.dma_start(bdmask[:], bdmask_d[:])
            maskqk = cpool.tile([128, 32, 64], BF16, name="maskqk")
            nc.sync.dma_start(maskqk[:], maskqk_d.rearrange("p (c k) -> p c k", c=32))
            for i in range(8):
                nc.sync.dma_start(xT[i][:, TB:N], xT_d[i * 128:(i + 1) * 128, TB:N])
            wout = []
            for p in range(NPAIR):
                t = ppool.tile([128, 1024], BF16, name=f"wout{p}", tag=f"wout{p}")
                nc.sync.dma_start(t[:], wout_d[p * 128:(p + 1) * 128, :])
                wout.append(t)

            # ---- PE warm-up: ~4us of dummy matmuls on ident so the HAM
            # clock gate opens while the weight DMAs are still in flight --
            warm = psq.tile([128, 512], F32, name="warm", tag="m")
            for w in range(36):
                nc.tensor.matmul(warm[:, 0:128], ident[:], ident[:],
                                 start=True, stop=True)

            # ---- cross-block running state ------------------------------
            runv0 = ppool.tile([1, 512], BF16, name="runv0", tag="runv0")
            nc.vector.memset(runv0[:], 0.0)
            runkT = ppool.tile([128, 4], F32, name="runkT", tag="runkT")
            nc.vector.memset(runkT[:], 0.0)
            cumkTb2 = []
            for j in range(2):
                tt_ = ppool.tile([128, 4, 32], BF16, name=f"cumkTb{j}",
                                 tag=f"cumkTb{j}")
                nc.vector.memset(tt_[:], 0.0)
                cumkTb2.append(tt_)

            # ---- staged emission with 1-block skew ----------------------
            def stage_a1(t, S):
                tok0 = t * TB

                # ---- qkT projection: 8 M-tiles (4 q pairs, 4 k pairs) ----
                qT_all = wpool.tile([128, 4, 8, 64], BF16, name=f"qT_all_{t}",
                                    tag="qT_all")
                kT_all = wpool.tile([128, 4, 8, 64], BF16, name=f"kT_all_{t}",
                                    tag="kT_all")
                for mt in range(8):
                    acc = psq.tile([128, 512], F32, name=f"qk_ps_{t}_{mt}", tag="m")
                    for i in range(8):
                        nc.tensor.matmul(
                            acc[:], wqk[i][:, mt * 128:(mt + 1) * 128],
                            xT[i][:, tok0:tok0 + TB],
                            start=(i == 0), stop=(i == 7))
                    dst = (qT_all if mt < 4 else kT_all)
                    nc.scalar.copy(dst[:, mt % 4, :, :],
                                   acc[:].rearrange("p (c k) -> p c k", c=8))

                # ---- v projection: 4 token tiles -------------------------
                v_sb = [None] * 4
                v_lo = [None] * 4  # odd chunk shifted to partitions 0-63
                for vt in range(4):
                    acc = psq.tile([128, 512], F32, name=f"v_ps_{t}_{vt}", tag="m")
                    for i in range(8):
                        nc.tensor.matmul(
                            acc[:], xT[i][:, tok0 + vt * 128:tok0 + (vt + 1) * 128],
                            wv[i][:], start=(i == 0), stop=(i == 7))
                    dst = wpool.tile([128, 512], BF16, name=f"v_{t}_{vt}", tag=f"v{vt}")
                    nc.vector.tensor_copy(dst[:], acc[:])
                    v_sb[vt] = dst
                    dst2 = wpool.tile([64, 512], BF16, name=f"vlo_{t}_{vt}",
                                      tag=f"vlo{vt}")
                    nc.gpsimd.tensor_copy(dst2[:], dst[64:128, :])
                    v_lo[vt] = dst2
                S.update(qT_all=qT_all, kT_all=kT_all, v_sb=v_sb, v_lo=v_lo)

            def stage_a2(t, S, prev):
                qT_all, kT_all, v_sb = S["qT_all"], S["kT_all"], S["v_sb"]
                # chunk means (x 0.5/64 folded into meanm)
                chunkv_ps = psq.tile([8, 512], F32, name=f"cv_ps_{t}", tag="m")
                for vt in range(4):
                    nc.tensor.matmul(chunkv_ps[:], meanm[:, vt * 8:(vt + 1) * 8],
                                     v_sb[vt][:], start=(vt == 0), stop=(vt == 3))
                chunkv = wpool.tile([8, 512], BF16, name=f"cv_{t}", tag="cv")
                nc.vector.tensor_copy(chunkv[:], chunkv_ps[:])

                # exclusive cumsum over chunks + carry; row 32 = new carry
                runv_prev = runv0[:] if prev is None else prev["runv"][:]
                cumv_ps = psq.tile([33, 512], F32, name=f"cumv_ps_{t}", tag="m")
                nc.tensor.matmul(cumv_ps[:], tri9[:], chunkv[:],
                                 start=True, stop=False)
                nc.tensor.matmul(cumv_ps[:], ones19[:], runv_prev,
                                 start=False, stop=True)
                cumv = wpool.tile([8, 512], BF16, name=f"cumv_{t}", tag="cumv")
                nc.vector.tensor_copy(cumv[:], cumv_ps[0:8, :])
                runv = wpool.tile([1, 512], BF16, name=f"runv_{t}", tag="runv")
                nc.vector.tensor_copy(runv[:], cumv_ps[32:33, :])

                # ---- cross-chunk cum_k + cross gate (moved up from stage_b
                # so the PE never stalls on this Vector chain) -------------
                ckT = wpool.tile([128, 4, 8], F32, name=f"ckT_{t}", tag="ckT")
                nc.vector.tensor_reduce(ckT[:], kT_all[:],
                                        axis=mybir.AxisListType.X,
                                        op=mybir.AluOpType.add)
                t1 = wpool.tile([128, 4, 8], F32, name=f"t1_{t}", tag="t1")
                nc.vector.tensor_copy(t1[:, :, 0:1], ckT[:, :, 0:1])
                nc.vector.tensor_add(t1[:, :, 1:8], ckT[:, :, 0:7], ckT[:, :, 1:8])
                t2 = wpool.tile([128, 4, 8], F32, name=f"t2_{t}", tag="t2")
                nc.vector.tensor_copy(t2[:, :, 0:2], t1[:, :, 0:2])
                nc.vector.tensor_add(t2[:, :, 2:8], t1[:, :, 0:6], t1[:, :, 2:8])
                incl = wpool.tile([128, 4, 8], F32, name=f"incl_{t}", tag="incl")
                nc.vector.tensor_copy(incl[:, :, 0:4], t2[:, :, 0:4])
                nc.vector.tensor_add(incl[:, :, 4:8], t2[:, :, 0:4], t2[:, :, 4:8])
                cumkT = wpool.tile([128, 4, 8], F32, name=f"cumkT_{t}", tag="cumkT")
                nc.vector.tensor_copy(cumkT[:, :, 0:1],
                                      runkT[:].broadcast_to([128, 4, 1]))
                nc.vector.tensor_add(cumkT[:, :, 1:8], incl[:, :, 0:7],
                                     runkT[:].broadcast_to([128, 4, 7]))
                nc.vector.tensor_add(runkT[:], runkT[:],
                                     incl[:, :, 7:8].rearrange("p a b -> p (a b)"))

                cumkTb = cumkTb2[t % 2]
                nc.vector.tensor_scalar_mul(cumkTb[:, :, 0:8], cumkT[:], 1.0 / 64)

                crA = psa.tile([128, 512], F32, name=f"crA_{t}", tag="m")
                crB = psa.tile([128, 512], F32, name=f"crB_{t}", tag="m")
                for p in range(NPAIR):
                    nc.tensor.matmul(
                        crA[32 * p:32 * p + 32, :], cumkTb[0:64, p, :],
                        qT_all[0:64, p, :, :].rearrange("p c k -> p (c k)"),
                        start=True, stop=True, skip_group_check=True,
                        tile_position=(0, 32 * p))
                    nc.tensor.matmul(
                        crB[32 * p:32 * p + 32, :], cumkTb[64:128, p, :],
                        qT_all[64:128, p, :, :].rearrange("p c k -> p (c k)"),
                        start=True, stop=True, skip_group_check=True,
                        tile_position=(64, 32 * p))
                cross = wpool.tile([128, 512], BF16, name=f"cross_{t}", tag="cross")
                nc.scalar.activation(cross[:], crA[:], AFT.Sigmoid)
                cross2 = wpool.tile([128, 512], BF16, name=f"cross2_{t}", tag="cross2")
                nc.scalar.activation(cross2[:], crB[:], AFT.Sigmoid)
                W8s = []
                for p in range(NPAIR):
                    W8 = wpool.tile([8, 1024], BF16, name=f"W8_{t}_{p}",
                                    tag=f"W8_{p}")
                    nc.vector.tensor_mul(W8[:, 0:512],
                                         cross[32 * p:32 * p + 8, :],
                                         bdmask[32 * p:32 * p + 8, :])
                    nc.vector.tensor_mul(W8[:, 512:1024],
                                         cross2[32 * p:32 * p + 8, :],
                                         bdmask[32 * p:32 * p + 8, :])
                    W8s.append(W8)

                S.update(cumv=cumv, runv=runv, W8s=W8s)

            def stage_b1(t, S):
                qT_all, kT_all = S["qT_all"], S["kT_all"]
                # scores + exp per pair (s8 psum freed quickly)
                E = wpool.tile([128, 32, 64], BF16, name=f"E_{t}", tag="E")
                denom = wpool.tile([128, 32], F32, name=f"den_{t}", tag="den")
                recip = wpool.tile([128, 32], F32, name=f"rec_{t}", tag="rec")
                for p in range(NPAIR):
                    s8 = psa.tile([128, 512], F32, name=f"s8_{t}_{p}", tag="m")
                    for c in range(8):
                        nc.tensor.matmul(
                            s8[0:64, c * 64:(c + 1) * 64],
                            qT_all[0:64, p, c, :], kT_all[0:64, p, c, :],
                            start=True, stop=True, tile_position=(0, 0))
                        nc.tensor.matmul(
                            s8[64:128, c * 64:(c + 1) * 64],
                            qT_all[64:128, p, c, :], kT_all[64:128, p, c, :],
                            start=True, stop=True, tile_position=(64, 64))
                    Ep = E[:, 8 * p:8 * (p + 1), :]
                    nc.scalar.activation(Ep,
                                         s8[:].rearrange("p (c k) -> p c k", c=8),
                                         AFT.Exp)
                    # per-pair softmax keeps the Vector chain short so the
                    # first transposes aren't gated on all four pairs
                    nc.vector.tensor_mul(Ep, Ep, maskqk[:, 0:8, :])
                    nc.vector.tensor_reduce(denom[:, 8 * p:8 * (p + 1)], Ep,
                                            axis=mybir.AxisListType.X,
                                            op=mybir.AluOpType.add)
                    nc.vector.reciprocal(recip[:, 8 * p:8 * (p + 1)],
                                         denom[:, 8 * p:8 * (p + 1)])
                    nc.vector.tensor_mul(
                        Ep, Ep,
                        recip[:, 8 * p:8 * (p + 1)].broadcast_to([128, 8, 64]))
                S["attn"] = E

            def stage_b2(t, S):
                v_sb, v_lo, cumv = S["v_sb"], S["v_lo"], S["cumv"]
                W8s, attn = S["W8s"], S["attn"]
                oTs = []
                etw = {}
                for pp in range(NPAIR + 1):
                    if pp < NPAIR:
                        p = pp
                        # transpose attn per chunk: [128q, 64k] -> [64k, 128q]
                        et1 = psa.tile([64, 512], BF16, name=f"et1_{t}_{p}", tag="m")
                        et2 = psa.tile([64, 512], BF16, name=f"et2_{t}_{p}", tag="m")
                        for c in range(8):
                            dst_ps = et1 if c < 4 else et2
                            nc.tensor.transpose(
                                dst_ps[:, (c % 4) * 128:(c % 4 + 1) * 128],
                                attn[:, 8 * p + c, :], ident[:])
                        ET = wpool.tile([64, 8, 128], BF16, name=f"ET_{t}_{p}",
                                        tag=f"ET{p % 2}")
                        nc.scalar.copy(ET[:, 0:4, :],
                                       et1[:].rearrange("p (c q) -> p c q", c=4))
                        nc.scalar.copy(ET[:, 4:8, :],
                                       et2[:].rearrange("p (c q) -> p c q", c=4))
                        etw[p] = (ET, W8s[p])

                    if pp >= 1:
                        p = pp - 1
                        ET, W8 = etw.pop(p)
                        # out_localT + cross term, accumulated in PSUM
                        o_ps = psa.tile([128, 512], F32, name=f"o_{t}_{p}", tag="m")
                        for c in range(8):
                            vt_, lo = c // 2, (c % 2)
                            vA = (v_sb[vt_] if lo == 0 else v_lo[vt_])
                            nc.tensor.matmul(
                                o_ps[0:64, c * 64:(c + 1) * 64],
                                vA[0:64, 2 * p * 64:(2 * p + 1) * 64],
                                ET[:, c, 0:64],
                                start=(c == 0), stop=False, tile_position=(0, 0),
                                skip_group_check=True)
                            nc.tensor.matmul(
                                o_ps[64:128, c * 64:(c + 1) * 64],
                                vA[0:64, (2 * p + 1) * 64:(2 * p + 2) * 64],
                                ET[:, c, 64:128],
                                start=(c == 0), stop=False, tile_position=(0, 64),
                                skip_group_check=True)
                        nc.tensor.matmul(o_ps[0:64, :],
                                         cumv[:, 2 * p * 64:(2 * p + 1) * 64],
                                         W8[:, 0:512], start=False, stop=True,
                                         tile_position=(0, 0), skip_group_check=True)
                        nc.tensor.matmul(o_ps[64:128, :],
                                         cumv[:, (2 * p + 1) * 64:(2 * p + 2) * 64],
                                         W8[:, 512:1024], start=False, stop=True,
                                         tile_position=(0, 64), skip_group_check=True)
                        oT = wpool.tile([128, 512], BF16, name=f"oT_{t}_{p}",
                                        tag=f"oT{p}")
                        nc.vector.tensor_copy(oT[:], o_ps[:])
                        oTs.append(oT)
                S["oTs"] = oTs

            def stage_c(t, S):
                tok0 = t * TB
                oTs = S["oTs"]
                # ---- out projection -------------------------------------
                # Full 2KB output rows per DMA (half the per-row descriptor
                # overhead) and a 2-tag fs rotation so the drains of
                # consecutive tiles overlap in different DMA queues.
                for tt in range(4):
                    fs = wpool.tile([128, 1024], BF16, name=f"fs_{t}_{tt}",
                                    tag=f"fs{tt % 2}")
                    for nt in range(2):
                        fo = psq.tile([128, 512], F32, name=f"fo_{t}_{nt}_{tt}",
                                     tag="m")
                        for p in range(NPAIR):
                            nc.tensor.matmul(
                                fo[:], oTs[p][:, tt * 128:(tt + 1) * 128],
                                wout[p][:, nt * 512:(nt + 1) * 512],
                                start=(p == 0), stop=(p == 3))
                        # steady state drains on Scalar (keeps Vector's
                        # softmax chain clear); at the tail split across
                        # both engines so neither queue backs up
                        if t < NBLK - 2 or nt == 0:
                            nc.scalar.copy(fs[:, nt * 512:(nt + 1) * 512], fo[:])
                        else:
                            nc.vector.tensor_copy(
                                fs[:, nt * 512:(nt + 1) * 512], fo[:])
                    nc.sync.dma_start(
                        out_d[tok0 + tt * 128:tok0 + tt * 128 + 64, :],
                        fs[0:64, :])
                    nc.sync.dma_start(
                        out_d[tok0 + tt * 128 + 64:tok0 + (tt + 1) * 128, :],
                        fs[64:128, :])


            # Emission order per iteration: a1(t) | b1(t-1) | c(t-2) |
            # a2(t) | b2(t-1).  The out-projection c sits between the
            # scores (b1) and the attn consumers (b2) in the PE queue so
            # the softmax Vector/Scalar chain is always covered by dense
            # matmul work; the final block's c runs as a tail.
            def fill(name, n):
                wfill = psq.tile([128, 512], F32, name=name, tag="m")
                for w in range(n):
                    nc.tensor.matmul(wfill[:, 0:128], ident[:], ident[:],
                                     start=True, stop=True)

            state = {}
            for t in range(NBLK + 1):
                if t < NBLK:
                    state[t] = {}
                    stage_a1(t, state[t])
                if 1 <= t <= NBLK:
                    stage_b1(t - 1, state[t - 1])
                if 2 <= t and t - 2 < NBLK - 1:
                    stage_c(t - 2, state[t - 2])
                elif t in (0, 1, 2, NBLK):
                    # nothing to cover the a2/b2 dependency chains at the
                    # edges — burn dummy matmuls to keep HAM warm and the
                    # PE queue moving
                    fill(f"wf_{t}", {0: 24, 1: 40, 2: 24, NBLK: 36}[t])
                if t < NBLK:
                    stage_a2(t, state[t], state.get(t - 1))
                if 1 <= t <= NBLK:
                    stage_b2(t - 1, state[t - 1])
            fill("wf_tail", 48)
            stage_c(NBLK - 1, state[NBLK - 1])

    nc.compile()
    return nc


def _consts():
    ident = np.eye(128, dtype=ml_dtypes.bfloat16)
    # causal in-chunk mask: row p (q = p % 64), col j valid if j <= q
    q = np.arange(128)[:, None] % 64
    j = np.arange(64)[None, :]
    maskqk = np.tile((j <= q).astype(np.float32), (1, 32)).astype(ml_dtypes.bfloat16)
    # chunk-mean matrices with 0.5 (cross factor) / 64 (mean) folded in
    meanm = np.zeros((128, 32), dtype=np.float32)
    for vt in range(4):
        meanm[0:64, vt * 8 + 2 * vt] = 0.5 / 64
        meanm[64:128, vt * 8 + 2 * vt + 1] = 0.5 / 64
    # tri9: [c, c'] = 1 if c < c' (exclusive cumsum); col 32 = all ones
    # (total -> new carry, at a 32-aligned PSUM partition)
    tri9 = np.zeros((8, 33), np.float32)
    tri9[:, 0:8] = np.triu(np.ones((8, 8), np.float32), 1)
    tri9[:, 32] = 1.0
    ones19 = np.ones((1, 33), np.float32)
    # block-diag mask for W8: [8, 1024]; cols 0-511 head A, 512-1023 head B
    c_ = np.arange(8)[:, None]
    col = np.arange(512)[None, :]
    bd = (col // 64 == c_).astype(np.float32)
    bdmask = np.zeros((128, 512), np.float32)
    for p_ in range(4):
        bdmask[32 * p_:32 * p_ + 8, :] = bd
    bf = ml_dtypes.bfloat16
    return {
        "ident": ident,
        "maskqk": maskqk,
        "meanm": meanm.astype(bf),
        "tri9": tri9.astype(bf),
        "ones19": ones19.astype(bf),
        "bdmask": bdmask.astype(bf),
    }


def _in_maps(x, W_qkv, W_out):
    bf = ml_dtypes.bfloat16
    consts = _consts()
    maps = []
    for c in range(N_CORES):
        b, hh = c // 2, c % 2
        heads = list(range(hh * HPC, (hh + 1) * HPC))
        xT = np.ascontiguousarray(x[b].T).astype(bf)
        qcols = np.concatenate(
            [W_qkv[:, 0 * DIM + h * D:(0 * DIM) + (h + 1) * D] for h in heads], axis=1)
        kcols = np.concatenate(
            [W_qkv[:, 1 * DIM + h * D:1 * DIM + (h + 1) * D] for h in heads], axis=1)
        vcols = np.concatenate(
            [W_qkv[:, 2 * DIM + h * D:2 * DIM + (h + 1) * D] for h in heads], axis=1)
        wqk = np.concatenate([qcols * SCALE, kcols], axis=1).astype(bf)
        wv = vcols.astype(bf)
        wout = np.concatenate([W_out[h * D:(h + 1) * D, :] for h in heads],
                              axis=0).astype(bf)
        m = {"xT": xT, "wqk": np.ascontiguousarray(wqk),
             "wv": np.ascontiguousarray(wv), "wout": np.ascontiguousarray(wout)}
        m.update(consts)
        maps.append(m)
    return maps


def kernel(x, W_qkv, W_out, _trace=False):
    if "nc" not in _cache:
        _cache["nc"] = _build()
    nc = _cache["nc"]
    maps = _in_maps(np.asarray(x, np.float32), np.asarray(W_qkv, np.float32),
                    np.asarray(W_out, np.float32))
    res = run_bass_kernel_spmd(nc, maps, core_ids=list(range(N_CORES)),
                               trace=_trace)
    _cache["last_result"] = res
    out = np.empty((B, N, DIM), np.float32)
    for b in range(B):
        out[b] = (res.results[2 * b]["out"].astype(np.float32)
                  + res.results[2 * b + 1]["out"].astype(np.float32))
    return out



# revision 35
# speedup vs baseline: 1.2367x; 1.2367x over previous
"""ChunkedLinearAttention Trainium2 kernel — 8-core SPMD.

Sharding: core c -> batch b = c//2, head-half hh = c%2 (8 of 16 heads).
Each core computes qkv projection for its heads, chunked local attention +
cross-chunk linear term, and a row-sharded out-projection producing a partial
[4096, 1024] output; host sums the two half partials per batch element.

All matmuls in bf16 (fp32 accumulate in PSUM).  Layouts:
  xT    [1024, 4096]  x[b] transposed (host-side), bf16
  qkT   [cols, tok]   computed on PE: lhsT=Wqk tile, rhs=xT tile
  v     [tok, vcols]  computed on PE: lhsT=xT tile, rhs=Wv
  per head-pair: qT/kT [128(2 heads x 64 dims), 8 chunks, 64 tok]
  scores S [128(2 heads x 64 q), 8, 64 k] via per-chunk matmuls packed with
  tile_position (head A rows 0-63 / head B rows 64-127)
  out_localT [128(2 heads x 64 dims), 512 tok] accumulated in PSUM, with the
  cross term added via a [K=8 chunks] matmul against cum_v.
"""

import sys

if "/opt/trn_rl_repo" not in sys.path:
    sys.path.insert(0, "/opt/trn_rl_repo")

import numpy as np
import ml_dtypes

import concourse.bacc as bacc
import concourse.tile as tile
import concourse.mybir as mybir
from concourse.bass_utils import run_bass_kernel_spmd

F32 = mybir.dt.float32
BF16 = mybir.dt.bfloat16
AFT = mybir.ActivationFunctionType

DIM, H, D, CS = 1024, 16, 64, 64
SCALE = D ** -0.5
B, N = 4, 4096
NBLK, TB = 8, 512          # token blocks
NC_CHUNKS = 8              # chunks per block
HPC = 8                    # heads per core
NPAIR = 4                  # head pairs per core
N_CORES = 8

_cache = {}


def _build():
    nc = bacc.Bacc("TRN2", target_bir_lowering=False, debug=False,
                   num_devices=N_CORES)

    # ---- DRAM I/O -------------------------------------------------------
    xT_d = nc.dram_tensor("xT", [DIM, N], BF16, kind="ExternalInput")
    wqk_d = nc.dram_tensor("wqk", [DIM, 1024], BF16, kind="ExternalInput")
    wv_d = nc.dram_tensor("wv", [DIM, 512], BF16, kind="ExternalInput")
    wout_d = nc.dram_tensor("wout", [512, DIM], BF16, kind="ExternalInput")
    ident_d = nc.dram_tensor("ident", [128, 128], BF16, kind="ExternalInput")
    maskqk_d = nc.dram_tensor("maskqk", [128, 2048], BF16, kind="ExternalInput")
    mean_d = nc.dram_tensor("meanm", [128, 32], BF16, kind="ExternalInput")
    tri9_d = nc.dram_tensor("tri9", [8, 33], BF16, kind="ExternalInput")
    ones19_d = nc.dram_tensor("ones19", [1, 33], BF16, kind="ExternalInput")
    bdmask_d = nc.dram_tensor("bdmask", [128, 512], BF16, kind="ExternalInput")
    # bf16 partials: host sums the two half-contraction partials in f32.
    out_d = nc.dram_tensor("out", [N, DIM], BF16, kind="ExternalOutput")

    with tile.TileContext(nc) as tc:
        with (
            tc.tile_pool(name="const", bufs=1) as cpool,
            tc.tile_pool(name="persist", bufs=1) as ppool,
            tc.tile_pool(name="work", bufs=2) as wpool,
            tc.tile_pool(name="psq", bufs=4, space="PSUM") as psq,
            tc.tile_pool(name="psa", bufs=4, space="PSUM") as psa,
        ):
            # ---- constants / weights into SBUF --------------------------
            # DMA issue order matters: the Sync engine serializes issue at
            # ~0.6us per DMA and transfers drain in issue order, so
            # block-0's dependencies (wqk + xT block 0) go FIRST; the small
            # constants are only needed mid-block and come later.  ident
            # leads so PE warm-up matmuls can run during the ~12us
            # bandwidth-bound weight load (HAM un-throttles after ~3.4us
            # of activity).
            ident = cpool.tile([128, 128], BF16, name="ident")
            nc.sync.dma_start(ident[:], ident_d[:])
            # Each DMA queue sustains only ~16B/ns, so a whole 256KB tile
            # on one queue takes ~16us.  Split the block-0-critical tiles
            # in half and issue from two engines in parallel so the
            # startup is bound by aggregate (not per-queue) bandwidth.
            wqk = []
            for i in range(8):
                t = ppool.tile([128, 1024], BF16, name=f"wqk{i}", tag=f"wqk{i}")
                nc.sync.dma_start(t[:], wqk_d[i * 128:(i + 1) * 128, :])
                wqk.append(t)
            xT = [ppool.tile([128, N], BF16, name=f"xT{i}", tag=f"xT{i}")
                  for i in range(8)]
            for i in range(8):
                nc.sync.dma_start(xT[i][:, 0:TB], xT_d[i * 128:(i + 1) * 128, 0:TB])
            wv = []
            for i in range(8):
                t = ppool.tile([128, 512], BF16, name=f"wv{i}", tag=f"wv{i}")
                nc.sync.dma_start(t[:], wv_d[i * 128:(i + 1) * 128, :])
                wv.append(t)
            meanm = cpool.tile([128, 32], BF16, name="meanm")
            nc.sync.dma_start(meanm[:], mean_d[:])
            tri9 = cpool.tile([8, 33], BF16, name="tri9")
            nc.sync.dma_start(tri9[:], tri9_d[:])
            ones19 = cpool.tile([1, 33], BF16, name="ones19")
            nc.sync.dma_start(ones19[:], ones19_d[:])
            bdmask = cpool.tile([128, 512], BF16, name="bdmask")
            nc.sync.dma_start(bdmask[:], bdmask_d[:])
            maskqk = cpool.tile([128, 32, 64], BF16, name="maskqk")
            nc.sync.dma_start(maskqk[:], maskqk_d.rearrange("p (c k) -> p c k", c=32))
            for i in range(8):
                nc.sync.dma_start(xT[i][:, TB:N], xT_d[i * 128:(i + 1) * 128, TB:N])
            wout = []
            for p in range(NPAIR):
                t = ppool.tile([128, 1024], BF16, name=f"wout{p}", tag=f"wout{p}")
                nc.sync.dma_start(t[:], wout_d[p * 128:(p + 1) * 128, :])
                wout.append(t)

            # ---- PE warm-up: ~4us of dummy matmuls on ident so the HAM
            # clock gate opens while the weight DMAs are still in flight --
            warm = psq.tile([128, 512], F32, name="warm", tag="m")
            for w in range(36):
                nc.tensor.matmul(warm[:, 0:128], ident[:], ident[:],
                                 start=True, stop=True)

            # ---- cross-block running state ------------------------------
            runv0 = ppool.tile([1, 512], BF16, name="runv0", tag="runv0")
            nc.vector.memset(runv0[:], 0.0)
            runkT = ppool.tile([128, 4], F32, name="runkT", tag="runkT")
            nc.vector.memset(runkT[:], 0.0)
            cumkTb2 = []
            for j in range(2):
                tt_ = ppool.tile([128, 4, 32], BF16, name=f"cumkTb{j}",
                                 tag=f"cumkTb{j}")
                nc.vector.memset(tt_[:], 0.0)
                cumkTb2.append(tt_)

            # ---- staged emission with 1-block skew ----------------------
            def stage_a1(t, S):
                tok0 = t * TB

                # ---- qkT projection: 8 M-tiles (4 q pairs, 4 k pairs) ----
                qT_all = wpool.tile([128, 4, 8, 64], BF16, name=f"qT_all_{t}",
                                    tag="qT_all")
                kT_all = wpool.tile([128, 4, 8, 64], BF16, name=f"kT_all_{t}",
                                    tag="kT_all")
                for mt in range(8):
                    acc = psq.tile([128, 512], F32, name=f"qk_ps_{t}_{mt}", tag="m")
                    for i in range(8):
                        nc.tensor.matmul(
                            acc[:], wqk[i][:, mt * 128:(mt + 1) * 128],
                            xT[i][:, tok0:tok0 + TB],
                            start=(i == 0), stop=(i == 7))
                    dst = (qT_all if mt < 4 else kT_all)
                    nc.scalar.copy(dst[:, mt % 4, :, :],
                                   acc[:].rearrange("p (c k) -> p c k", c=8))

                # ---- v projection: 4 token tiles -------------------------
                v_sb = [None] * 4
                v_lo = [None] * 4  # odd chunk shifted to partitions 0-63
                for vt in range(4):
                    acc = psq.tile([128, 512], F32, name=f"v_ps_{t}_{vt}", tag="m")
                    for i in range(8):
                        nc.tensor.matmul(
                            acc[:], xT[i][:, tok0 + vt * 128:tok0 + (vt + 1) * 128],
                            wv[i][:], start=(i == 0), stop=(i == 7))
                    dst = wpool.tile([128, 512], BF16, name=f"v_{t}_{vt}", tag=f"v{vt}")
                    nc.vector.tensor_copy(dst[:], acc[:])
                    v_sb[vt] = dst
                    dst2 = wpool.tile([64, 512], BF16, name=f"vlo_{t}_{vt}",
                                      tag=f"vlo{vt}")
                    nc.gpsimd.tensor_copy(dst2[:], dst[64:128, :])
                    v_lo[vt] = dst2
                S.update(qT_all=qT_all, kT_all=kT_all, v_sb=v_sb, v_lo=v_lo)

            def stage_a2(t, S, prev):
                qT_all, kT_all, v_sb = S["qT_all"], S["kT_all"], S["v_sb"]
                # chunk means (x 0.5/64 folded into meanm)
                chunkv_ps = psq.tile([8, 512], F32, name=f"cv_ps_{t}", tag="m")
                for vt in range(4):
                    nc.tensor.matmul(chunkv_ps[:], meanm[:, vt * 8:(vt + 1) * 8],
                                     v_sb[vt][:], start=(vt == 0), stop=(vt == 3))
                chunkv = wpool.tile([8, 512], BF16, name=f"cv_{t}", tag="cv")
                nc.vector.tensor_copy(chunkv[:], chunkv_ps[:])

                # exclusive cumsum over chunks + carry; row 32 = new carry
                runv_prev = runv0[:] if prev is None else prev["runv"][:]
                cumv_ps = psq.tile([33, 512], F32, name=f"cumv_ps_{t}", tag="m")
                nc.tensor.matmul(cumv_ps[:], tri9[:], chunkv[:],
                                 start=True, stop=False)
                nc.tensor.matmul(cumv_ps[:], ones19[:], runv_prev,
                                 start=False, stop=True)
                cumv = wpool.tile([8, 512], BF16, name=f"cumv_{t}", tag="cumv")
                nc.vector.tensor_copy(cumv[:], cumv_ps[0:8, :])
                runv = wpool.tile([1, 512], BF16, name=f"runv_{t}", tag="runv")
                nc.vector.tensor_copy(runv[:], cumv_ps[32:33, :])

                # ---- cross-chunk cum_k + cross gate (moved up from stage_b
                # so the PE never stalls on this Vector chain) -------------
                ckT = wpool.tile([128, 4, 8], F32, name=f"ckT_{t}", tag="ckT")
                nc.vector.tensor_reduce(ckT[:], kT_all[:],
                                        axis=mybir.AxisListType.X,
                                        op=mybir.AluOpType.add)
                t1 = wpool.tile([128, 4, 8], F32, name=f"t1_{t}", tag="t1")
                nc.vector.tensor_copy(t1[:, :, 0:1], ckT[:, :, 0:1])
                nc.vector.tensor_add(t1[:, :, 1:8], ckT[:, :, 0:7], ckT[:, :, 1:8])
                t2 = wpool.tile([128, 4, 8], F32, name=f"t2_{t}", tag="t2")
                nc.vector.tensor_copy(t2[:, :, 0:2], t1[:, :, 0:2])
                nc.vector.tensor_add(t2[:, :, 2:8], t1[:, :, 0:6], t1[:, :, 2:8])
                incl = wpool.tile([128, 4, 8], F32, name=f"incl_{t}", tag="incl")
                nc.vector.tensor_copy(incl[:, :, 0:4], t2[:, :, 0:4])
                nc.vector.tensor_add(incl[:, :, 4:8], t2[:, :, 0:4], t2[:, :, 4:8])
                cumkT = wpool.tile([128, 4, 8], F32, name=f"cumkT_{t}", tag="cumkT")
                nc.vector.tensor_copy(cumkT[:, :, 0:1],
                                      runkT[:].broadcast_to([128, 4, 1]))
                nc.vector.tensor_add(cumkT[:, :, 1:8], incl[:, :, 0:7],
                                     runkT[:].broadcast_to([128, 4, 7]))
                nc.vector.tensor_add(runkT[:], runkT[:],
                                     incl[:, :, 7:8].rearrange("p a b -> p (a b)"))

                cumkTb = cumkTb2[t % 2]
                nc.vector.tensor_scalar_mul(cumkTb[:, :, 0:8], cumkT[:], 1.0 / 64)

                crA = psa.tile([128, 512], F32, name=f"crA_{t}", tag="m")
                crB = psa.tile([128, 512], F32, name=f"crB_{t}", tag="m")
                for p in range(NPAIR):
                    nc.tensor.matmul(
                        crA[32 * p:32 * p + 32, :], cumkTb[0:64, p, :],
                        qT_all[0:64, p, :, :].rearrange("p c k -> p (c k)"),
                        start=True, stop=True, skip_group_check=True,
                        tile_position=(0, 32 * p))
                    nc.tensor.matmul(
                        crB[32 * p:32 * p + 32, :], cumkTb[64:128, p, :],
                        qT_all[64:128, p, :, :].rearrange("p c k -> p (c k)"),
                        start=True, stop=True, skip_group_check=True,
                        tile_position=(64, 32 * p))
                cross = wpool.tile([128, 512], BF16, name=f"cross_{t}", tag="cross")
                nc.scalar.activation(cross[:], crA[:], AFT.Sigmoid)
                cross2 = wpool.tile([128, 512], BF16, name=f"cross2_{t}", tag="cross2")
                nc.scalar.activation(cross2[:], crB[:], AFT.Sigmoid)
                W8s = []
                for p in range(NPAIR):
                    W8 = wpool.tile([8, 1024], BF16, name=f"W8_{t}_{p}",
                                    tag=f"W8_{p}")
                    nc.vector.tensor_mul(W8[:, 0:512],
                                         cross[32 * p:32 * p + 8, :],
                                         bdmask[32 * p:32 * p + 8, :])
                    nc.vector.tensor_mul(W8[:, 512:1024],
                                         cross2[32 * p:32 * p + 8, :],
                                         bdmask[32 * p:32 * p + 8, :])
                    W8s.append(W8)

                S.update(cumv=cumv, runv=runv, W8s=W8s)

            def stage_b1(t, S):
                qT_all, kT_all = S["qT_all"], S["kT_all"]
                # scores + exp per pair (s8 psum freed quickly)
                E = wpool.tile([128, 32, 64], BF16, name=f"E_{t}", tag="E")
                for p in range(NPAIR):
                    s8 = psa.tile([128, 512], F32, name=f"s8_{t}_{p}", tag="m")
                    for c in range(8):
                        nc.tensor.matmul(
                            s8[0:64, c * 64:(c + 1) * 64],
                            qT_all[0:64, p, c, :], kT_all[0:64, p, c, :],
                            start=True, stop=True, tile_position=(0, 0))
                        nc.tensor.matmul(
                            s8[64:128, c * 64:(c + 1) * 64],
                            qT_all[64:128, p, c, :], kT_all[64:128, p, c, :],
                            start=True, stop=True, tile_position=(64, 64))
                    nc.scalar.activation(E[:, 8 * p:8 * (p + 1), :],
                                         s8[:].rearrange("p (c k) -> p c k", c=8),
                                         AFT.Exp)
                nc.vector.tensor_mul(E[:], E[:], maskqk[:])
                denom = wpool.tile([128, 32], F32, name=f"den_{t}", tag="den")
                nc.vector.tensor_reduce(denom[:], E[:], axis=mybir.AxisListType.X,
                                        op=mybir.AluOpType.add)
                recip = wpool.tile([128, 32], F32, name=f"rec_{t}", tag="rec")
                nc.vector.reciprocal(recip[:], denom[:])
                attn = E
                nc.vector.tensor_mul(attn[:], E[:],
                                     recip[:].broadcast_to([128, 32, 64]))
                S["attn"] = attn

            def stage_b2(t, S):
                v_sb, v_lo, cumv = S["v_sb"], S["v_lo"], S["cumv"]
                W8s, attn = S["W8s"], S["attn"]
                oTs = []
                etw = {}
                for pp in range(NPAIR + 1):
                    if pp < NPAIR:
                        p = pp
                        # transpose attn per chunk: [128q, 64k] -> [64k, 128q]
                        et1 = psa.tile([64, 512], BF16, name=f"et1_{t}_{p}", tag="m")
                        et2 = psa.tile([64, 512], BF16, name=f"et2_{t}_{p}", tag="m")
                        for c in range(8):
                            dst_ps = et1 if c < 4 else et2
                            nc.tensor.transpose(
                                dst_ps[:, (c % 4) * 128:(c % 4 + 1) * 128],
                                attn[:, 8 * p + c, :], ident[:])
                        ET = wpool.tile([64, 8, 128], BF16, name=f"ET_{t}_{p}",
                                        tag=f"ET{p % 2}")
                        nc.scalar.copy(ET[:, 0:4, :],
                                       et1[:].rearrange("p (c q) -> p c q", c=4))
                        nc.scalar.copy(ET[:, 4:8, :],
                                       et2[:].rearrange("p (c q) -> p c q", c=4))
                        etw[p] = (ET, W8s[p])

                    if pp >= 1:
                        p = pp - 1
                        ET, W8 = etw.pop(p)
                        # out_localT + cross term, accumulated in PSUM
                        o_ps = psa.tile([128, 512], F32, name=f"o_{t}_{p}", tag="m")
                        for c in range(8):
                            vt_, lo = c // 2, (c % 2)
                            vA = (v_sb[vt_] if lo == 0 else v_lo[vt_])
                            nc.tensor.matmul(
                                o_ps[0:64, c * 64:(c + 1) * 64],
                                vA[0:64, 2 * p * 64:(2 * p + 1) * 64],
                                ET[:, c, 0:64],
                                start=(c == 0), stop=False, tile_position=(0, 0),
                                skip_group_check=True)
                            nc.tensor.matmul(
                                o_ps[64:128, c * 64:(c + 1) * 64],
                                vA[0:64, (2 * p + 1) * 64:(2 * p + 2) * 64],
                                ET[:, c, 64:128],
                                start=(c == 0), stop=False, tile_position=(0, 64),
                                skip_group_check=True)
                        nc.tensor.matmul(o_ps[0:64, :],
                                         cumv[:, 2 * p * 64:(2 * p + 1) * 64],
                                         W8[:, 0:512], start=False, stop=True,
                                         tile_position=(0, 0), skip_group_check=True)
                        nc.tensor.matmul(o_ps[64:128, :],
                                         cumv[:, (2 * p + 1) * 64:(2 * p + 2) * 64],
                                         W8[:, 512:1024], start=False, stop=True,
                                         tile_position=(0, 64), skip_group_check=True)
                        oT = wpool.tile([128, 512], BF16, name=f"oT_{t}_{p}",
                                        tag=f"oT{p}")
                        nc.vector.tensor_copy(oT[:], o_ps[:])
                        oTs.append(oT)
                S["oTs"] = oTs

            def stage_c(t, S):
                tok0 = t * TB
                oTs = S["oTs"]
                # ---- out projection -------------------------------------
                # Full 2KB output rows per DMA (half the per-row descriptor
                # overhead) and a 2-tag fs rotation so the drains of
                # consecutive tiles overlap in different DMA queues.
                for tt in range(4):
                    fs = wpool.tile([128, 1024], BF16, name=f"fs_{t}_{tt}",
                                    tag=f"fs{tt % 2}")
                    for nt in range(2):
                        fo = psq.tile([128, 512], F32, name=f"fo_{t}_{nt}_{tt}",
                                     tag="m")
                        for p in range(NPAIR):
                            nc.tensor.matmul(
                                fo[:], oTs[p][:, tt * 128:(tt + 1) * 128],
                                wout[p][:, nt * 512:(nt + 1) * 512],
                                start=(p == 0), stop=(p == 3))
                        # steady state drains on Scalar (keeps Vector's
                        # softmax chain clear); at the tail split across
                        # both engines so neither queue backs up
                        if t < NBLK - 2 or nt == 0:
                            nc.scalar.copy(fs[:, nt * 512:(nt + 1) * 512], fo[:])
                        else:
                            nc.vector.tensor_copy(
                                fs[:, nt * 512:(nt + 1) * 512], fo[:])
                    nc.sync.dma_start(
                        out_d[tok0 + tt * 128:tok0 + tt * 128 + 64, :],
                        fs[0:64, :])
                    nc.sync.dma_start(
                        out_d[tok0 + tt * 128 + 64:tok0 + (tt + 1) * 128, :],
                        fs[64:128, :])


            # Emission order per iteration: a1(t) | b1(t-1) | c(t-2) |
            # a2(t) | b2(t-1).  The out-projection c sits between the
            # scores (b1) and the attn consumers (b2) in the PE queue so
            # the softmax Vector/Scalar chain is always covered by dense
            # matmul work; the final block's c runs as a tail.
            def fill(name, n):
                wfill = psq.tile([128, 512], F32, name=name, tag="m")
                for w in range(n):
                    nc.tensor.matmul(wfill[:, 0:128], ident[:], ident[:],
                                     start=True, stop=True)

            state = {}
            for t in range(NBLK + 1):
                if t < NBLK:
                    state[t] = {}
                    stage_a1(t, state[t])
                if 1 <= t <= NBLK:
                    stage_b1(t - 1, state[t - 1])
                if 2 <= t and t - 2 < NBLK - 1:
                    stage_c(t - 2, state[t - 2])
                elif t in (0, 1, 2, NBLK):
                    # nothing to cover the a2/b2 dependency chains at the
                    # edges — burn dummy matmuls to keep HAM warm and the
                    # PE queue moving
                    fill(f"wf_{t}", {0: 24, 1: 40, 2: 24, NBLK: 36}[t])
                if t < NBLK:
                    stage_a2(t, state[t], state.get(t - 1))
                if 1 <= t <= NBLK:
                    stage_b2(t - 1, state[t - 1])
            fill("wf_tail", 48)
            stage_c(NBLK - 1, state[NBLK - 1])

    nc.compile()
    return nc


def _consts():
    ident = np.eye(128, dtype=ml_dtypes.bfloat16)
    # causal in-chunk mask: row p (q = p % 64), col j valid if j <= q
    q = np.arange(128)[:, None] % 64
    j = np.arange(64)[None, :]
    maskqk = np.tile((j <= q).astype(np.float32), (1, 32)).astype(ml_dtypes.bfloat16)
    # chunk-mean matrices with 0.5 (cross factor) / 64 (mean) folded in
    meanm = np.zeros((128, 32), dtype=np.float32)
    for vt in range(4):
        meanm[0:64, vt * 8 + 2 * vt] = 0.5 / 64
        meanm[64:128, vt * 8 + 2 * vt + 1] = 0.5 / 64
    # tri9: [c, c'] = 1 if c < c' (exclusive cumsum); col 32 = all ones
    # (total -> new carry, at a 32-aligned PSUM partition)
    tri9 = np.zeros((8, 33), np.float32)
    tri9[:, 0:8] = np.triu(np.ones((8, 8), np.float32), 1)
    tri9[:, 32] = 1.0
    ones19 = np.ones((1, 33), np.float32)
    # block-diag mask for W8: [8, 1024]; cols 0-511 head A, 512-1023 head B
    c_ = np.arange(8)[:, None]
    col = np.arange(512)[None, :]
    bd = (col // 64 == c_).astype(np.float32)
    bdmask = np.zeros((128, 512), np.float32)
    for p_ in range(4):
        bdmask[32 * p_:32 * p_ + 8, :] = bd
    bf = ml_dtypes.bfloat16
    return {
        "ident": ident,
        "maskqk": maskqk,
        "meanm": meanm.astype(bf),
        "tri9": tri9.astype(bf),
        "ones19": ones19.astype(bf),
        "bdmask": bdmask.astype(bf),
    }


def _in_maps(x, W_qkv, W_out):
    bf = ml_dtypes.bfloat16
    consts = _consts()
    maps = []
    for c in range(N_CORES):
        b, hh = c // 2, c % 2
        heads = list(range(hh * HPC, (hh + 1) * HPC))
        xT = np.ascontiguousarray(x[b].T).astype(bf)
        qcols = np.concatenate(
            [W_qkv[:, 0 * DIM + h * D:(0 * DIM) + (h + 1) * D] for h in heads], axis=1)
        kcols = np.concatenate(
            [W_qkv[:, 1 * DIM + h * D:1 * DIM + (h + 1) * D] for h in heads], axis=1)
        vcols = np.concatenate(
            [W_qkv[:, 2 * DIM + h * D:2 * DIM + (h + 1) * D] for h in heads], axis=1)
        wqk = np.concatenate([qcols * SCALE, kcols], axis=1).astype(bf)
        wv = vcols.astype(bf)
        wout = np.concatenate([W_out[h * D:(h + 1) * D, :] for h in heads],
                              axis=0).astype(bf)
        m = {"xT": xT, "wqk": np.ascontiguousarray(wqk),
             "wv": np.ascontiguousarray(wv), "wout": np.ascontiguousarray(wout)}
        m.update(consts)
        maps.append(m)
    return maps


def kernel(x, W_qkv, W_out, _trace=False):
    if "nc" not in _cache:
        _cache["nc"] = _build()
    nc = _cache["nc"]
    maps = _in_maps(np.asarray(x, np.float32), np.asarray(W_qkv, np.float32),
                    np.asarray(W_out, np.float32))
    res = run_bass_kernel_spmd(nc, maps, core_ids=list(range(N_CORES)),
                               trace=_trace)
    _cache["last_result"] = res
    out = np.empty((B, N, DIM), np.float32)
    for b in range(B):
        out[b] = (res.results[2 * b]["out"].astype(np.float32)
                  + res.results[2 * b + 1]["out"].astype(np.float32))
    return out



# revision 36
# speedup vs baseline: 1.2521x; 1.0124x over previous
"""ChunkedLinearAttention Trainium2 kernel — 8-core SPMD.

Sharding: core c -> batch b = c//2, head-half hh = c%2 (8 of 16 heads).
Each core computes qkv projection for its heads, chunked local attention +
cross-chunk linear term, and a row-sharded out-projection producing a partial
[4096, 1024] output; host sums the two half partials per batch element.

All matmuls in bf16 (fp32 accumulate in PSUM).  Layouts:
  xT    [1024, 4096]  x[b] transposed (host-side), bf16
  qkT   [cols, tok]   computed on PE: lhsT=Wqk tile, rhs=xT tile
  v     [tok, vcols]  computed on PE: lhsT=xT tile, rhs=Wv
  per head-pair: qT/kT [128(2 heads x 64 dims), 8 chunks, 64 tok]
  scores S [128(2 heads x 64 q), 8, 64 k] via per-chunk matmuls packed with
  tile_position (head A rows 0-63 / head B rows 64-127)
  out_localT [128(2 heads x 64 dims), 512 tok] accumulated in PSUM, with the
  cross term added via a [K=8 chunks] matmul against cum_v.
"""

import sys

if "/opt/trn_rl_repo" not in sys.path:
    sys.path.insert(0, "/opt/trn_rl_repo")

import numpy as np
import ml_dtypes

import concourse.bacc as bacc
import concourse.tile as tile
import concourse.mybir as mybir
from concourse.bass_utils import run_bass_kernel_spmd

F32 = mybir.dt.float32
BF16 = mybir.dt.bfloat16
AFT = mybir.ActivationFunctionType

DIM, H, D, CS = 1024, 16, 64, 64
SCALE = D ** -0.5
B, N = 4, 4096
NBLK, TB = 8, 512          # token blocks
NC_CHUNKS = 8              # chunks per block
HPC = 8                    # heads per core
NPAIR = 4                  # head pairs per core
N_CORES = 8

_cache = {}


def _build():
    nc = bacc.Bacc("TRN2", target_bir_lowering=False, debug=False,
                   num_devices=N_CORES)

    # ---- DRAM I/O -------------------------------------------------------
    xT_d = nc.dram_tensor("xT", [DIM, N], BF16, kind="ExternalInput")
    wqk_d = nc.dram_tensor("wqk", [DIM, 1024], BF16, kind="ExternalInput")
    wv_d = nc.dram_tensor("wv", [DIM, 512], BF16, kind="ExternalInput")
    wout_d = nc.dram_tensor("wout", [512, DIM], BF16, kind="ExternalInput")
    ident_d = nc.dram_tensor("ident", [128, 128], BF16, kind="ExternalInput")
    maskqk_d = nc.dram_tensor("maskqk", [128, 2048], BF16, kind="ExternalInput")
    mean_d = nc.dram_tensor("meanm", [128, 32], BF16, kind="ExternalInput")
    tri9_d = nc.dram_tensor("tri9", [8, 33], BF16, kind="ExternalInput")
    ones19_d = nc.dram_tensor("ones19", [1, 33], BF16, kind="ExternalInput")
    bdmask_d = nc.dram_tensor("bdmask", [128, 512], BF16, kind="ExternalInput")
    # bf16 partials: host sums the two half-contraction partials in f32.
    out_d = nc.dram_tensor("out", [N, DIM], BF16, kind="ExternalOutput")

    with tile.TileContext(nc) as tc:
        with (
            tc.tile_pool(name="const", bufs=1) as cpool,
            tc.tile_pool(name="persist", bufs=1) as ppool,
            tc.tile_pool(name="work", bufs=2) as wpool,
            tc.tile_pool(name="psq", bufs=4, space="PSUM") as psq,
            tc.tile_pool(name="psa", bufs=4, space="PSUM") as psa,
        ):
            # ---- constants / weights into SBUF --------------------------
            # DMA issue order matters: the Sync engine serializes issue at
            # ~0.6us per DMA and transfers drain in issue order, so
            # block-0's dependencies (wqk + xT block 0) go FIRST; the small
            # constants are only needed mid-block and come later.  ident
            # leads so PE warm-up matmuls can run during the ~12us
            # bandwidth-bound weight load (HAM un-throttles after ~3.4us
            # of activity).
            ident = cpool.tile([128, 128], BF16, name="ident")
            nc.sync.dma_start(ident[:], ident_d[:])
            # Each DMA queue sustains only ~16B/ns, so a whole 256KB tile
            # on one queue takes ~16us.  Split the block-0-critical tiles
            # in half and issue from two engines in parallel so the
            # startup is bound by aggregate (not per-queue) bandwidth.
            wqk = []
            for i in range(8):
                t = ppool.tile([128, 1024], BF16, name=f"wqk{i}", tag=f"wqk{i}")
                nc.sync.dma_start(t[:], wqk_d[i * 128:(i + 1) * 128, :])
                wqk.append(t)
            xT = [ppool.tile([128, N], BF16, name=f"xT{i}", tag=f"xT{i}")
                  for i in range(8)]
            for i in range(8):
                nc.sync.dma_start(xT[i][:, 0:TB], xT_d[i * 128:(i + 1) * 128, 0:TB])
            wv = []
            for i in range(8):
                t = ppool.tile([128, 512], BF16, name=f"wv{i}", tag=f"wv{i}")
                nc.sync.dma_start(t[:], wv_d[i * 128:(i + 1) * 128, :])
                wv.append(t)
            meanm = cpool.tile([128, 32], BF16, name="meanm")
            nc.sync.dma_start(meanm[:], mean_d[:])
            tri9 = cpool.tile([8, 33], BF16, name="tri9")
            nc.sync.dma_start(tri9[:], tri9_d[:])
            ones19 = cpool.tile([1, 33], BF16, name="ones19")
            nc.sync.dma_start(ones19[:], ones19_d[:])
            bdmask = cpool.tile([128, 512], BF16, name="bdmask")
            nc.sync.dma_start(bdmask[:], bdmask_d[:])
            maskqk = cpool.tile([128, 32, 64], BF16, name="maskqk")
            nc.sync.dma_start(maskqk[:], maskqk_d.rearrange("p (c k) -> p c k", c=32))
            for i in range(8):
                nc.sync.dma_start(xT[i][:, TB:N], xT_d[i * 128:(i + 1) * 128, TB:N])
            wout = []
            for p in range(NPAIR):
                t = ppool.tile([128, 1024], BF16, name=f"wout{p}", tag=f"wout{p}")
                nc.sync.dma_start(t[:], wout_d[p * 128:(p + 1) * 128, :])
                wout.append(t)

            # ---- PE warm-up: ~4us of dummy matmuls on ident so the HAM
            # clock gate opens while the weight DMAs are still in flight --
            warm = psq.tile([128, 512], F32, name="warm", tag="m")
            for w in range(160):
                nc.tensor.matmul(warm[:, 0:128], ident[:], ident[:],
                                 start=True, stop=True)

            # ---- cross-block running state ------------------------------
            runv0 = ppool.tile([1, 512], BF16, name="runv0", tag="runv0")
            nc.vector.memset(runv0[:], 0.0)
            runkT = ppool.tile([128, 4], F32, name="runkT", tag="runkT")
            nc.vector.memset(runkT[:], 0.0)
            cumkTb2 = []
            for j in range(2):
                tt_ = ppool.tile([128, 4, 32], BF16, name=f"cumkTb{j}",
                                 tag=f"cumkTb{j}")
                nc.vector.memset(tt_[:], 0.0)
                cumkTb2.append(tt_)

            # ---- staged emission with 1-block skew ----------------------
            def stage_a1(t, S):
                tok0 = t * TB

                # ---- qkT projection: 8 M-tiles (4 q pairs, 4 k pairs) ----
                qT_all = wpool.tile([128, 4, 8, 64], BF16, name=f"qT_all_{t}",
                                    tag="qT_all")
                kT_all = wpool.tile([128, 4, 8, 64], BF16, name=f"kT_all_{t}",
                                    tag="kT_all")
                for mt in range(8):
                    acc = psq.tile([128, 512], F32, name=f"qk_ps_{t}_{mt}", tag="m")
                    for i in range(8):
                        nc.tensor.matmul(
                            acc[:], wqk[i][:, mt * 128:(mt + 1) * 128],
                            xT[i][:, tok0:tok0 + TB],
                            start=(i == 0), stop=(i == 7))
                    dst = (qT_all if mt < 4 else kT_all)
                    nc.scalar.copy(dst[:, mt % 4, :, :],
                                   acc[:].rearrange("p (c k) -> p c k", c=8))

                # ---- v projection: 4 token tiles -------------------------
                v_sb = [None] * 4
                v_lo = [None] * 4  # odd chunk shifted to partitions 0-63
                for vt in range(4):
                    acc = psq.tile([128, 512], F32, name=f"v_ps_{t}_{vt}", tag="m")
                    for i in range(8):
                        nc.tensor.matmul(
                            acc[:], xT[i][:, tok0 + vt * 128:tok0 + (vt + 1) * 128],
                            wv[i][:], start=(i == 0), stop=(i == 7))
                    dst = wpool.tile([128, 512], BF16, name=f"v_{t}_{vt}", tag=f"v{vt}")
                    nc.vector.tensor_copy(dst[:], acc[:])
                    v_sb[vt] = dst
                    dst2 = wpool.tile([64, 512], BF16, name=f"vlo_{t}_{vt}",
                                      tag=f"vlo{vt}")
                    nc.gpsimd.tensor_copy(dst2[:], dst[64:128, :])
                    v_lo[vt] = dst2
                S.update(qT_all=qT_all, kT_all=kT_all, v_sb=v_sb, v_lo=v_lo)

            def stage_a2(t, S, prev):
                qT_all, kT_all, v_sb = S["qT_all"], S["kT_all"], S["v_sb"]
                # chunk means (x 0.5/64 folded into meanm)
                chunkv_ps = psq.tile([8, 512], F32, name=f"cv_ps_{t}", tag="m")
                for vt in range(4):
                    nc.tensor.matmul(chunkv_ps[:], meanm[:, vt * 8:(vt + 1) * 8],
                                     v_sb[vt][:], start=(vt == 0), stop=(vt == 3))
                chunkv = wpool.tile([8, 512], BF16, name=f"cv_{t}", tag="cv")
                nc.vector.tensor_copy(chunkv[:], chunkv_ps[:])

                # exclusive cumsum over chunks + carry; row 32 = new carry
                runv_prev = runv0[:] if prev is None else prev["runv"][:]
                cumv_ps = psq.tile([33, 512], F32, name=f"cumv_ps_{t}", tag="m")
                nc.tensor.matmul(cumv_ps[:], tri9[:], chunkv[:],
                                 start=True, stop=False)
                nc.tensor.matmul(cumv_ps[:], ones19[:], runv_prev,
                                 start=False, stop=True)
                cumv = wpool.tile([8, 512], BF16, name=f"cumv_{t}", tag="cumv")
                nc.vector.tensor_copy(cumv[:], cumv_ps[0:8, :])
                runv = wpool.tile([1, 512], BF16, name=f"runv_{t}", tag="runv")
                nc.vector.tensor_copy(runv[:], cumv_ps[32:33, :])

                # ---- cross-chunk cum_k + cross gate (moved up from stage_b
                # so the PE never stalls on this Vector chain) -------------
                ckT = wpool.tile([128, 4, 8], F32, name=f"ckT_{t}", tag="ckT")
                nc.vector.tensor_reduce(ckT[:], kT_all[:],
                                        axis=mybir.AxisListType.X,
                                        op=mybir.AluOpType.add)
                t1 = wpool.tile([128, 4, 8], F32, name=f"t1_{t}", tag="t1")
                nc.vector.tensor_copy(t1[:, :, 0:1], ckT[:, :, 0:1])
                nc.vector.tensor_add(t1[:, :, 1:8], ckT[:, :, 0:7], ckT[:, :, 1:8])
                t2 = wpool.tile([128, 4, 8], F32, name=f"t2_{t}", tag="t2")
                nc.vector.tensor_copy(t2[:, :, 0:2], t1[:, :, 0:2])
                nc.vector.tensor_add(t2[:, :, 2:8], t1[:, :, 0:6], t1[:, :, 2:8])
                incl = wpool.tile([128, 4, 8], F32, name=f"incl_{t}", tag="incl")
                nc.vector.tensor_copy(incl[:, :, 0:4], t2[:, :, 0:4])
                nc.vector.tensor_add(incl[:, :, 4:8], t2[:, :, 0:4], t2[:, :, 4:8])
                cumkT = wpool.tile([128, 4, 8], F32, name=f"cumkT_{t}", tag="cumkT")
                nc.vector.tensor_copy(cumkT[:, :, 0:1],
                                      runkT[:].broadcast_to([128, 4, 1]))
                nc.vector.tensor_add(cumkT[:, :, 1:8], incl[:, :, 0:7],
                                     runkT[:].broadcast_to([128, 4, 7]))
                nc.vector.tensor_add(runkT[:], runkT[:],
                                     incl[:, :, 7:8].rearrange("p a b -> p (a b)"))

                cumkTb = cumkTb2[t % 2]
                nc.vector.tensor_scalar_mul(cumkTb[:, :, 0:8], cumkT[:], 1.0 / 64)

                crA = psa.tile([128, 512], F32, name=f"crA_{t}", tag="m")
                crB = psa.tile([128, 512], F32, name=f"crB_{t}", tag="m")
                for p in range(NPAIR):
                    nc.tensor.matmul(
                        crA[32 * p:32 * p + 32, :], cumkTb[0:64, p, :],
                        qT_all[0:64, p, :, :].rearrange("p c k -> p (c k)"),
                        start=True, stop=True, skip_group_check=True,
                        tile_position=(0, 32 * p))
                    nc.tensor.matmul(
                        crB[32 * p:32 * p + 32, :], cumkTb[64:128, p, :],
                        qT_all[64:128, p, :, :].rearrange("p c k -> p (c k)"),
                        start=True, stop=True, skip_group_check=True,
                        tile_position=(64, 32 * p))
                cross = wpool.tile([128, 512], BF16, name=f"cross_{t}", tag="cross")
                nc.scalar.activation(cross[:], crA[:], AFT.Sigmoid)
                cross2 = wpool.tile([128, 512], BF16, name=f"cross2_{t}", tag="cross2")
                nc.scalar.activation(cross2[:], crB[:], AFT.Sigmoid)
                W8s = []
                for p in range(NPAIR):
                    W8 = wpool.tile([8, 1024], BF16, name=f"W8_{t}_{p}",
                                    tag=f"W8_{p}")
                    nc.vector.tensor_mul(W8[:, 0:512],
                                         cross[32 * p:32 * p + 8, :],
                                         bdmask[32 * p:32 * p + 8, :])
                    nc.vector.tensor_mul(W8[:, 512:1024],
                                         cross2[32 * p:32 * p + 8, :],
                                         bdmask[32 * p:32 * p + 8, :])
                    W8s.append(W8)

                S.update(cumv=cumv, runv=runv, W8s=W8s)

            def stage_b1(t, S):
                qT_all, kT_all = S["qT_all"], S["kT_all"]
                # scores + exp per pair (s8 psum freed quickly)
                E = wpool.tile([128, 32, 64], BF16, name=f"E_{t}", tag="E")
                for p in range(NPAIR):
                    s8 = psa.tile([128, 512], F32, name=f"s8_{t}_{p}", tag="m")
                    for c in range(8):
                        nc.tensor.matmul(
                            s8[0:64, c * 64:(c + 1) * 64],
                            qT_all[0:64, p, c, :], kT_all[0:64, p, c, :],
                            start=True, stop=True, tile_position=(0, 0))
                        nc.tensor.matmul(
                            s8[64:128, c * 64:(c + 1) * 64],
                            qT_all[64:128, p, c, :], kT_all[64:128, p, c, :],
                            start=True, stop=True, tile_position=(64, 64))
                    nc.scalar.activation(E[:, 8 * p:8 * (p + 1), :],
                                         s8[:].rearrange("p (c k) -> p c k", c=8),
                                         AFT.Exp)
                nc.vector.tensor_mul(E[:], E[:], maskqk[:])
                denom = wpool.tile([128, 32], F32, name=f"den_{t}", tag="den")
                nc.vector.tensor_reduce(denom[:], E[:], axis=mybir.AxisListType.X,
                                        op=mybir.AluOpType.add)
                recip = wpool.tile([128, 32], F32, name=f"rec_{t}", tag="rec")
                nc.vector.reciprocal(recip[:], denom[:])
                attn = E
                nc.vector.tensor_mul(attn[:], E[:],
                                     recip[:].broadcast_to([128, 32, 64]))
                S["attn"] = attn

            def stage_b2(t, S):
                v_sb, v_lo, cumv = S["v_sb"], S["v_lo"], S["cumv"]
                W8s, attn = S["W8s"], S["attn"]
                oTs = []
                etw = {}
                for pp in range(NPAIR + 1):
                    if pp < NPAIR:
                        p = pp
                        # transpose attn per chunk: [128q, 64k] -> [64k, 128q]
                        et1 = psa.tile([64, 512], BF16, name=f"et1_{t}_{p}", tag="m")
                        et2 = psa.tile([64, 512], BF16, name=f"et2_{t}_{p}", tag="m")
                        for c in range(8):
                            dst_ps = et1 if c < 4 else et2
                            nc.tensor.transpose(
                                dst_ps[:, (c % 4) * 128:(c % 4 + 1) * 128],
                                attn[:, 8 * p + c, :], ident[:])
                        ET = wpool.tile([64, 8, 128], BF16, name=f"ET_{t}_{p}",
                                        tag=f"ET{p % 2}")
                        nc.scalar.copy(ET[:, 0:4, :],
                                       et1[:].rearrange("p (c q) -> p c q", c=4))
                        nc.scalar.copy(ET[:, 4:8, :],
                                       et2[:].rearrange("p (c q) -> p c q", c=4))
                        etw[p] = (ET, W8s[p])

                    if pp >= 1:
                        p = pp - 1
                        ET, W8 = etw.pop(p)
                        # out_localT + cross term, accumulated in PSUM
                        o_ps = psa.tile([128, 512], F32, name=f"o_{t}_{p}", tag="m")
                        for c in range(8):
                            vt_, lo = c // 2, (c % 2)
                            vA = (v_sb[vt_] if lo == 0 else v_lo[vt_])
                            nc.tensor.matmul(
                                o_ps[0:64, c * 64:(c + 1) * 64],
                                vA[0:64, 2 * p * 64:(2 * p + 1) * 64],
                                ET[:, c, 0:64],
                                start=(c == 0), stop=False, tile_position=(0, 0),
                                skip_group_check=True)
                            nc.tensor.matmul(
                                o_ps[64:128, c * 64:(c + 1) * 64],
                                vA[0:64, (2 * p + 1) * 64:(2 * p + 2) * 64],
                                ET[:, c, 64:128],
                                start=(c == 0), stop=False, tile_position=(0, 64),
                                skip_group_check=True)
                        nc.tensor.matmul(o_ps[0:64, :],
                                         cumv[:, 2 * p * 64:(2 * p + 1) * 64],
                                         W8[:, 0:512], start=False, stop=True,
                                         tile_position=(0, 0), skip_group_check=True)
                        nc.tensor.matmul(o_ps[64:128, :],
                                         cumv[:, (2 * p + 1) * 64:(2 * p + 2) * 64],
                                         W8[:, 512:1024], start=False, stop=True,
                                         tile_position=(0, 64), skip_group_check=True)
                        oT = wpool.tile([128, 512], BF16, name=f"oT_{t}_{p}",
                                        tag=f"oT{p}")
                        # the last block's oT drains go to Scalar: Vector
                        # is the critical path at the tail
                        if t == NBLK - 1:
                            nc.scalar.copy(oT[:], o_ps[:])
                        else:
                            nc.vector.tensor_copy(oT[:], o_ps[:])
                        oTs.append(oT)
                S["oTs"] = oTs

            def stage_c(t, S):
                tok0 = t * TB
                oTs = S["oTs"]
                # ---- out projection -------------------------------------
                # Full 2KB output rows per DMA (half the per-row descriptor
                # overhead) and a 2-tag fs rotation so the drains of
                # consecutive tiles overlap in different DMA queues.
                for tt in range(4):
                    fs = wpool.tile([128, 1024], BF16, name=f"fs_{t}_{tt}",
                                    tag=f"fs{tt % 2}")
                    for nt in range(2):
                        fo = psq.tile([128, 512], F32, name=f"fo_{t}_{nt}_{tt}",
                                     tag="m")
                        for p in range(NPAIR):
                            nc.tensor.matmul(
                                fo[:], oTs[p][:, tt * 128:(tt + 1) * 128],
                                wout[p][:, nt * 512:(nt + 1) * 512],
                                start=(p == 0), stop=(p == 3))
                        # steady state drains on Scalar (keeps Vector's
                        # softmax chain clear); at the tail split across
                        # both engines so neither queue backs up
                        if t < NBLK - 2 or nt == 0:
                            nc.scalar.copy(fs[:, nt * 512:(nt + 1) * 512], fo[:])
                        else:
                            nc.vector.tensor_copy(
                                fs[:, nt * 512:(nt + 1) * 512], fo[:])
                    nc.sync.dma_start(
                        out_d[tok0 + tt * 128:tok0 + tt * 128 + 64, :],
                        fs[0:64, :])
                    nc.sync.dma_start(
                        out_d[tok0 + tt * 128 + 64:tok0 + (tt + 1) * 128, :],
                        fs[64:128, :])


            # Emission order per iteration: a1(t) | b1(t-1) | c(t-2) |
            # a2(t) | b2(t-1).  The out-projection c sits between the
            # scores (b1) and the attn consumers (b2) in the PE queue so
            # the softmax Vector/Scalar chain is always covered by dense
            # matmul work; the final block's c runs as a tail.
            def fill(name, n):
                wfill = psq.tile([128, 512], F32, name=name, tag="m")
                for w in range(n):
                    nc.tensor.matmul(wfill[:, 0:128], ident[:], ident[:],
                                     start=True, stop=True)

            state = {}
            for t in range(NBLK + 1):
                if t < NBLK:
                    state[t] = {}
                    stage_a1(t, state[t])
                if 1 <= t <= NBLK:
                    stage_b1(t - 1, state[t - 1])
                if 2 <= t and t - 2 < NBLK - 1:
                    stage_c(t - 2, state[t - 2])
                elif t in (0, 1, 2, NBLK):
                    # nothing to cover the a2/b2 dependency chains at the
                    # edges — burn dummy matmuls to keep HAM warm and the
                    # PE queue moving
                    fill(f"wf_{t}", {0: 36, 1: 72, 2: 48, NBLK: 36}[t])
                if t < NBLK:
                    stage_a2(t, state[t], state.get(t - 1))
                if 1 <= t <= NBLK:
                    stage_b2(t - 1, state[t - 1])
            fill("wf_tail", 48)
            stage_c(NBLK - 1, state[NBLK - 1])

    nc.compile()
    return nc


def _consts():
    ident = np.eye(128, dtype=ml_dtypes.bfloat16)
    # causal in-chunk mask: row p (q = p % 64), col j valid if j <= q
    q = np.arange(128)[:, None] % 64
    j = np.arange(64)[None, :]
    maskqk = np.tile((j <= q).astype(np.float32), (1, 32)).astype(ml_dtypes.bfloat16)
    # chunk-mean matrices with 0.5 (cross factor) / 64 (mean) folded in
    meanm = np.zeros((128, 32), dtype=np.float32)
    for vt in range(4):
        meanm[0:64, vt * 8 + 2 * vt] = 0.5 / 64
        meanm[64:128, vt * 8 + 2 * vt + 1] = 0.5 / 64
    # tri9: [c, c'] = 1 if c < c' (exclusive cumsum); col 32 = all ones
    # (total -> new carry, at a 32-aligned PSUM partition)
    tri9 = np.zeros((8, 33), np.float32)
    tri9[:, 0:8] = np.triu(np.ones((8, 8), np.float32), 1)
    tri9[:, 32] = 1.0
    ones19 = np.ones((1, 33), np.float32)
    # block-diag mask for W8: [8, 1024]; cols 0-511 head A, 512-1023 head B
    c_ = np.arange(8)[:, None]
    col = np.arange(512)[None, :]
    bd = (col // 64 == c_).astype(np.float32)
    bdmask = np.zeros((128, 512), np.float32)
    for p_ in range(4):
        bdmask[32 * p_:32 * p_ + 8, :] = bd
    bf = ml_dtypes.bfloat16
    return {
        "ident": ident,
        "maskqk": maskqk,
        "meanm": meanm.astype(bf),
        "tri9": tri9.astype(bf),
        "ones19": ones19.astype(bf),
        "bdmask": bdmask.astype(bf),
    }


def _in_maps(x, W_qkv, W_out):
    bf = ml_dtypes.bfloat16
    consts = _consts()
    maps = []
    for c in range(N_CORES):
        b, hh = c // 2, c % 2
        heads = list(range(hh * HPC, (hh + 1) * HPC))
        xT = np.ascontiguousarray(x[b].T).astype(bf)
        qcols = np.concatenate(
            [W_qkv[:, 0 * DIM + h * D:(0 * DIM) + (h + 1) * D] for h in heads], axis=1)
        kcols = np.concatenate(
            [W_qkv[:, 1 * DIM + h * D:1 * DIM + (h + 1) * D] for h in heads], axis=1)
        vcols = np.concatenate(
            [W_qkv[:, 2 * DIM + h * D:2 * DIM + (h + 1) * D] for h in heads], axis=1)
        wqk = np.concatenate([qcols * SCALE, kcols], axis=1).astype(bf)
        wv = vcols.astype(bf)
        wout = np.concatenate([W_out[h * D:(h + 1) * D, :] for h in heads],
                              axis=0).astype(bf)
        m = {"xT": xT, "wqk": np.ascontiguousarray(wqk),
             "wv": np.ascontiguousarray(wv), "wout": np.ascontiguousarray(wout)}
        m.update(consts)
        maps.append(m)
    return maps


def kernel(x, W_qkv, W_out, _trace=False):
    if "nc" not in _cache:
        _cache["nc"] = _build()
    nc = _cache["nc"]
    maps = _in_maps(np.asarray(x, np.float32), np.asarray(W_qkv, np.float32),
                    np.asarray(W_out, np.float32))
    res = run_bass_kernel_spmd(nc, maps, core_ids=list(range(N_CORES)),
                               trace=_trace)
    _cache["last_result"] = res
    out = np.empty((B, N, DIM), np.float32)
    for b in range(B):
        out[b] = (res.results[2 * b]["out"].astype(np.float32)
                  + res.results[2 * b + 1]["out"].astype(np.float32))
    return out

